# revision 11
# baseline (speedup 1.0000x reference)
"""Trainium2 Bass kernel for nn_BinarizedLinear (ES population binary matvec).

Computes, for each direction d: out[d, o] = (sum_i W[d,o,i] * x[d,i]) > bias[d,o]
with W in {0,1} (f32), x in {0,1} (bool), bias f32.

Strategy (v5 — bit-packed popcount; the original nn.Module stores these
weights bit-packed in int64 words, so the host packs them the same way):
  - Host packs W bits 8-per-byte: HBM traffic drops 32x vs f32
    (134 MiB -> 4.2 MiB per core; the f32 baseline was DMA-bound ~350 us).
  - Layout, per group of 2 directions: u16[j, e, o] = byte(B=2j) |
    byte(B=2j+1)<<8 where byte B holds input bits i = 8B..8B+7 of output
    row o of direction 2g+e. One partition owns byte pair (2j, 2j+1).
  - DVE (the Pool engine has no integer ALU, so the whole SWAR chain runs
    on DVE; tensor_scalar imm ops run in 4x_2p mode, tensor_tensor in
    2x_1p; two directions per instruction to halve fixed overheads):
      z = Wp & x_dup      (TT; x stored as duplicated u16 pairs so the
                           broadcast AP keeps a stride-1 last dim)
      s = (z >> 1) & 0x5555                  (TS imm)
      u = z - s           (TT) -> SWAR pair-counts {0,1,2} in 2-bit fields
      plane_f = (u << {3,1} | >> {1,3}) & 0x1818   (TS imm, 4 planes)
    Plane byte values {0x00,0x08,0x10} read as fp8e4m3 are {0, 2^-6, 2^-5}:
    exactly linear in the count (normal fp8 range, no subnormals).
  - PE reduces each plane over j with fp8 DoubleRow matmuls (2 k-tiles of
    128 partitions = all 256 bytes per contraction): stationary is a
    one-hot column of 64.0 at slot d*4+c, so psum[32, 512] row d*4+c
    accumulates act[d, c*512:(c+1)*512] = sum of counts exactly
    (64 * 2^-6 * count = count; integer f32 accumulation, bit-exact).
    The walrus ldw-opt pass merges the redundant LDWEIGHTS between the 4
    consecutive same-stationary matmuls (128 -> 32 loads).
  - Tail: one DVE is_gt psum vs bias (host pre-arranged [32, 512]) -> u8 out.
"""

from contextlib import ExitStack

import numpy as np

import concourse.bass as bass
import concourse.bass_utils as _bu
from concourse import mybir
from concourse.bass_utils import run_bass_kernel_spmd

# Walrus tuning: let the backend merge the 4x-redundant LDWEIGHTS between
# consecutive same-stationary matmuls.
_orig_run_command = _bu.run_command


def _patched_run_command(cmd, *a, **kw):
    if cmd and isinstance(cmd, list) and "walrus_driver" in str(cmd[0]):
        cmd = [
            "--enable-ldw-opt=true" if c == "--enable-ldw-opt=false" else c
            for c in cmd
        ]
    return _orig_run_command(cmd, *a, **kw)


if _bu.run_command.__name__ != "_patched_run_command":
    _bu.run_command = _patched_run_command

N_CORES = 8
D_TOT, OUT, IN = 64, 2048, 2048
D = D_TOT // N_CORES  # 8 directions per core
P = 128
NG = D // 2           # 4 groups of 2 directions
NCH = 4               # o-chunks of 512 (psum free width)
CHW = OUT // NCH      # 512
NPL = 4               # pair planes per direction
OPG = 3 + NPL         # DVE ops per group (z, s, u, 4 extracts)
# plane f covers bit-pair (2f, 2f+1) of each byte; the shift moves the
# 2-bit field to bits 3-4 (fp8e4 values {0, 2^-6, 2^-5}, linear in count).
PLANE_SHIFTS = [(mybir.AluOpType.logical_shift_left, 3),
                (mybir.AluOpType.logical_shift_left, 1),
                (mybir.AluOpType.logical_shift_right, 1),
                (mybir.AluOpType.logical_shift_right, 3)]
PLANE_MASK = 0x1818
STAT_HOT = 32         # one-hot column position in statw


def build_program() -> bass.Bass:
    f32 = mybir.dt.float32
    f8 = mybir.dt.float8e4
    u16 = mybir.dt.uint16
    u8 = mybir.dt.uint8
    Alu = mybir.AluOpType
    DR = mybir.MatmulPerfMode.DoubleRow

    nc = bass.Bass()
    wp = nc.declare_dram_parameter("wp", [NG, P, 2, OUT], u16, isOutput=False)
    xaux = nc.declare_dram_parameter("xaux", [P, 2 * D], u16, isOutput=False)
    b = nc.declare_dram_parameter("b", [4 * D, CHW], f32, isOutput=False)
    o = nc.declare_dram_parameter("o", [4 * D, CHW], u8, isOutput=True)

    psum = nc.alloc_psum_tensor("psum", [4 * D, CHW], f32)

    with ExitStack() as ctx:
        wbufs = [
            ctx.enter_context(nc.sbuf_tensor(f"w{r}", [P, 2, OUT], u16))
            for r in range(2)
        ]
        zb = ctx.enter_context(nc.sbuf_tensor("zb", [P, 2, OUT], u16))
        sb = ctx.enter_context(nc.sbuf_tensor("sb", [P, 2, OUT], u16))
        ub = ctx.enter_context(nc.sbuf_tensor("ub", [P, 2, OUT], u16))
        planes = [
            ctx.enter_context(
                nc.sbuf_tensor(f"pl{r}", [P, NPL, 2, OUT], u16)
            )
            for r in range(2)
        ]
        xa_sb = ctx.enter_context(nc.sbuf_tensor("xa_sb", [P, 2 * D], u16))
        statw = ctx.enter_context(nc.sbuf_tensor("statw", [P, 2, 64], f8))
        bias_sb = ctx.enter_context(nc.sbuf_tensor("bias_sb", [4 * D, CHW], f32))
        outc = ctx.enter_context(nc.sbuf_tensor("outc", [4 * D, CHW], u8))

        block = ctx.enter_context(nc.Block())
        wsem = [
            ctx.enter_context(nc.semaphore(f"wsem{g}")) for g in range(NG)
        ]
        xsem = ctx.enter_context(nc.semaphore("xsem"))
        bsem = ctx.enter_context(nc.semaphore("bsem"))
        dv_sem = ctx.enter_context(nc.semaphore("dv_sem"))
        pe_sem = ctx.enter_context(nc.semaphore("pe_sem"))
        st_sem = ctx.enter_context(nc.semaphore("st_sem"))
        cmp_sem = ctx.enter_context(nc.semaphore("cmp_sem"))
        out_sem = ctx.enter_context(nc.semaphore("out_sem"))

        @block.sync
        def _(sp):
            for g in range(NG):
                if g >= 2:
                    # wbuf slot g%2 free once z of group g-2 was read
                    sp.wait_ge(dv_sem, OPG * (g - 2) + 1)
                sp.dma_start(out=wbufs[g % 2][:], in_=wp[g]).then_inc(
                    wsem[g], 16
                )

        @block.scalar
        def _(act):
            act.dma_start(out=xa_sb[:], in_=xaux[:]).then_inc(xsem, 16)
            act.dma_start(out=bias_sb[:], in_=b[:]).then_inc(bsem, 16)
            act.wait_ge(cmp_sem, 1)
            act.dma_start(out=o[:], in_=outc[:]).then_inc(out_sem, 16)
            act.wait_ge(out_sem, 16)

        @block.gpsimd
        def _(gp):
            gp.memset(statw[:], 0.0).then_inc(st_sem, 1)
            gp.wait_ge(st_sem, 1)
            gp.memset(statw[:, :, STAT_HOT:STAT_HOT + 1], 64.0).then_inc(
                st_sem, 1
            )

        @block.vector
        def _(dve):
            for g in range(NG):
                r = g % 2
                if g == 0:
                    dve.wait_ge(xsem, 16)
                dve.wait_ge(wsem[g], 16)
                # x operand [P, 2 (dir), 1, 2] -> bcast [P, 2, OUT//2, 2]
                xbc = xa_sb[:, 4 * g:4 * g + 4].rearrange(
                    "p (d a b) -> p d a b", a=1, b=2
                ).broadcast_to([P, 2, OUT // 2, 2])
                dve.tensor_tensor(
                    out=zb[:].rearrange("p d (a b) -> p d a b", b=2),
                    in0=wbufs[r][:].rearrange("p d (a b) -> p d a b", b=2),
                    in1=xbc,
                    op=Alu.bitwise_and,
                ).then_inc(dv_sem, 1)
                dve.wait_ge(dv_sem, OPG * g + 1)
                dve.tensor_scalar(
                    out=sb[:], in0=zb[:],
                    scalar1=1, scalar2=0x5555,
                    op0=Alu.logical_shift_right, op1=Alu.bitwise_and,
                ).then_inc(dv_sem, 1)
                dve.wait_ge(dv_sem, OPG * g + 2)
                dve.tensor_tensor(
                    out=ub[:], in0=zb[:], in1=sb[:], op=Alu.subtract
                ).then_inc(dv_sem, 1)
                dve.wait_ge(dv_sem, OPG * g + 3)
                if g >= 2:
                    # planes slot reuse: PE consumed group g-2's planes
                    dve.wait_ge(pe_sem, g - 1)
                for f in range(NPL):
                    op0, sh = PLANE_SHIFTS[f]
                    dve.tensor_scalar(
                        out=planes[r][:, f, :, :], in0=ub[:],
                        scalar1=sh, scalar2=PLANE_MASK,
                        op0=op0, op1=Alu.bitwise_and,
                    ).then_inc(dv_sem, 1)
            dve.wait_ge(pe_sem, NG)
            dve.wait_ge(bsem, 16)
            dve.tensor_tensor(
                out=outc[:], in0=psum[:], in1=bias_sb[:], op=Alu.is_gt
            ).then_inc(cmp_sem, 1)

        @block.tensor
        def _(pe):
            pe.wait_ge(st_sem, 2)
            for g in range(NG):
                r = g % 2
                # fp8 view: [P, NPL, 2 (dir), 2*OUT], byte index 2*o + pair
                pl8 = planes[r][:].bitcast(f8)
                for e in range(2):
                    d = 2 * g + e
                    for c in range(NCH):
                        hot = STAT_HOT - (4 * d + c)
                        lhsT = statw[:, :, hot:hot + 32]
                        for f in range(NPL):
                            if c == 0 and e == 0:
                                pe.wait_ge(dv_sem, OPG * g + 4 + f)
                            rhs = pl8[:, f, e, :].rearrange(
                                "p (o t) -> p t o", t=2
                            )[:, :, c * CHW:(c + 1) * CHW]
                            mm = pe.matmul(
                                out=psum[:],
                                lhsT=lhsT,
                                rhs=rhs,
                                start=(d == 0 and c == 0 and f == 0),
                                stop=(d == D - 1 and c == NCH - 1
                                      and f == NPL - 1),
                                perf_mode=DR,
                            )
                            if e == 1 and c == NCH - 1 and f == NPL - 1:
                                mm.then_inc(pe_sem, 1)

    return nc


_prog = None


def _get_prog() -> bass.Bass:
    global _prog
    if _prog is None:
        _prog = build_program()
    return _prog


def _pack_core(w, x, bias):
    """Build one core's input map from its [D, OUT, IN] f32 / [D, IN] bool /
    [D, OUT] f32 shard."""
    pb = np.packbits(
        np.ascontiguousarray(w) != 0, axis=-1, bitorder="little"
    )  # [D, OUT, IN/8]
    # u16[g, j, e, o] = byte(2j, o) | byte(2j+1, o) << 8 of direction 2g+e
    arr = pb.transpose(0, 2, 1).reshape(D, P, 2, OUT)  # [d, j, bpair, o]
    wp16 = np.ascontiguousarray(arr.transpose(0, 1, 3, 2)).view(np.uint16)
    wp16 = wp16.reshape(NG, 2, P, OUT).transpose(0, 2, 1, 3)  # [g, j, e, o]
    wp16 = np.ascontiguousarray(wp16)

    xb = np.packbits(np.ascontiguousarray(x), axis=-1, bitorder="little")
    xw = np.ascontiguousarray(xb.reshape(D, P, 2)).view(np.uint16).reshape(D, P)
    # duplicated pairs: cols (2d, 2d+1) both hold xw[d]
    xaux = np.repeat(xw.T, 2, axis=1).astype(np.uint16)  # [P, 2D]
    xaux = np.ascontiguousarray(xaux)

    br = np.ascontiguousarray(bias.astype(np.float32).reshape(D * NCH, CHW))
    return {"wp": wp16, "xaux": xaux, "b": br}


def make_in_maps(weight_noise, x, bias_noise):
    in_maps = []
    for c in range(N_CORES):
        sl = slice(c * D, (c + 1) * D)
        in_maps.append(_pack_core(weight_noise[sl], x[sl], bias_noise[sl]))
    return in_maps


def kernel(**inputs) -> np.ndarray:
    nc = _get_prog()
    in_maps = make_in_maps(
        inputs["weight_noise"], inputs["x"], inputs["bias_noise"]
    )
    res = run_bass_kernel_spmd(nc, in_maps, list(range(N_CORES)))
    outs = [res.results[c]["o"].reshape(D, OUT) for c in range(N_CORES)]
    return np.concatenate(outs, axis=0).astype(bool)
